# revision 3
# baseline (speedup 1.0000x reference)
"""Bi-directional WKV (RWKV-style) kernel for Trainium2, 8-core batch-parallel.

Math per (b, t, c):
    tf_b     = sigmoid(sum(time_emb[b]))
    decay_bc = exp(-exp(w_c)) * (0.5 + 0.5*tf_b)
    ek   = exp(k);  ekv = ek * v
    nf/df = forward inclusive scans of ekv/ek over t;  nb/db = backward
    num  = nf + nb + (e^u - 1)*ekv
    den  = df + db + (e^u - 1)*ek
    out  = sigmoid(r) * num/den * (0.8 + 0.2*tf_b)

Engine assignment (one batch element per core, [T=2048, C=2048] f32):

The DVE's four tensor_tensor_scan ops run at 2 cycles/elem regardless of
dtype (bf16 packed, materialized data0, Pool-engine offload: all measured
no-ops or compiler crashes), so everything else is pushed off the DVE:

  - num/den are assembled ON THE TENSOR ENGINE: per 128x128 block, three
    accumulating matmuls (start/stop flags) into one PSUM bank transpose
    nf/nb/ekv (df/db/ek) into natural [T, C] layout; the third uses a
    diag(e^u - 1) moving matrix, folding the c1 scale in for free.  bf16
    stationary data keeps these 1-pass (f32 matmul lowers to 2 passes).
    is_transpose=True cannot be used here: the transpose datapath ignores
    the moving matrix and cannot scale.
  - scan outputs and ek/ekv are written as bf16 (scan state stays fp32
    internally), halving their SBUF traffic and feeding the bf16 matmuls.
  - r is consumed in natural layout (never transposed); the sigmoid
    factor and the division run in log domain on the Scalar engine with
    one activation-table set (Ln/Exp only, zero table reloads):
       inv = exp(-(ln(den) + ln(1+e^-r)) + ln(0.8+0.2*tf))
  - the s = ln(den) + l1p add runs as a gpsimd SBUF->SBUF DMA with
    accum_op=add (software-DGE compute), not on the DVE.
  - log-domain tensors stay f32 (bf16's 0.4% relative error on ln(den)
    ~ 30 would exponentiate to ~12% output error).

DVE/ctile ends at 4 scans + 4 ekv mults + 4 final mults (~20us), with
Act ~12us, PE ~14us, DMA ~11us - wall ~404us vs the 569us baseline.
"""

import numpy as np
from contextlib import ExitStack

import concourse.bass as bass
import concourse.bacc as bacc
import concourse.tile as tile
from concourse import mybir
from concourse.bass_utils import run_bass_kernel_spmd
from concourse.masks import make_identity

from concourse.hw_specs import get_activation_tables


def _pin_act_tables():
    # Confine the ACT table-set choice (see baseline note): every
    # Exp/Ln/Copy and the one Sigmoid can be served by two sets; hiding
    # the alternatives stops the table-load pass from bouncing.
    tabs = get_activation_tables("gen3")
    keep = {"natural_log_exp_and_others", "sigmoid_and_friends"}
    for name in list(tabs):
        if name not in keep:
            tabs[name] = set()


_pin_act_tables()

B, T, C, TD = 8, 2048, 2048, 512
P = 128
N_CORES = 8
f32 = mybir.dt.float32
bf16 = mybir.dt.bfloat16
ALU = mybir.AluOpType
AF = mybir.ActivationFunctionType


def _body(tc, out, r, k, v, w, u, te, T_, C_, TD_):
    nc = tc.nc
    NT = T_ // P   # t-blocks per ctile (16)
    NCT = C_ // P  # ctiles (16)
    CH = 512       # psum chunk width (one bank)
    BPC = CH // P  # 128-blocks per chunk (4)
    NCH = T_ // CH  # chunks per ctile (4)

    with ExitStack() as ctx:
        consts = ctx.enter_context(tc.tile_pool(name="consts", bufs=1))
        slabs = ctx.enter_context(tc.tile_pool(name="slabs", bufs=2))
        cbuf = ctx.enter_context(tc.tile_pool(name="cbuf", bufs=2))
        scano = ctx.enter_context(tc.tile_pool(name="scano", bufs=2))
        small = ctx.enter_context(tc.tile_pool(name="small", bufs=1))
        psin = ctx.enter_context(tc.tile_pool(name="psin", bufs=2, space="PSUM"))
        psnum = ctx.enter_context(tc.tile_pool(name="psnum", bufs=2, space="PSUM"))
        psden = ctx.enter_context(tc.tile_pool(name="psden", bufs=2, space="PSUM"))

        ident = consts.tile([P, P], f32)
        make_identity(nc, ident[:])
        identb = consts.tile([P, P], bf16)
        nc.vector.tensor_scalar(out=identb[:], in0=ident[:], scalar1=1.0,
                                scalar2=None, op0=ALU.mult)

        # ---- per-batch time factor on all partitions ----
        te_t = consts.tile([P, TD_], f32)
        te_b = bass.AP(tensor=te.tensor, offset=te.offset, ap=[[0, P]] + list(te.ap))
        nc.gpsimd.dma_start(out=te_t[:], in_=te_b)
        ssum = consts.tile([P, 1], f32)
        nc.vector.tensor_reduce(out=ssum[:], in_=te_t[:], axis=mybir.AxisListType.X,
                                op=ALU.add)
        tf = consts.tile([P, 1], f32)
        nc.scalar.activation(out=tf[:], in_=ssum[:], func=AF.Sigmoid)
        scale_b = consts.tile([P, 1], f32)   # 0.8 + 0.2*tf
        nc.vector.tensor_scalar(out=scale_b[:], in0=tf[:], scalar1=0.2, scalar2=0.8,
                                op0=ALU.mult, op1=ALU.add)
        htf = consts.tile([P, 1], f32)       # 0.5 + 0.5*tf
        nc.vector.tensor_scalar(out=htf[:], in0=tf[:], scalar1=0.5, scalar2=0.5,
                                op0=ALU.mult, op1=ALU.add)
        lnscale = consts.tile([P, 1], f32)   # ln(0.8 + 0.2*tf)
        nc.scalar.activation(out=lnscale[:], in_=scale_b[:], func=AF.Ln)

        # ---- per-channel constants [128, NCT]: channel = j*128 + p ----
        wt = consts.tile([P, NCT], f32)
        nc.gpsimd.dma_start(out=wt[:], in_=w.rearrange("(j p) -> p j", p=P))
        ut = consts.tile([P, NCT], f32)
        nc.gpsimd.dma_start(out=ut[:], in_=u.rearrange("(j p) -> p j", p=P))
        ew = consts.tile([P, NCT], f32)
        nc.scalar.activation(out=ew[:], in_=wt[:], func=AF.Exp)          # e^w
        dec0 = consts.tile([P, NCT], f32)
        nc.scalar.activation(out=dec0[:], in_=ew[:], func=AF.Exp, scale=-1.0)
        decay = consts.tile([P, NCT], f32)
        nc.vector.tensor_scalar(out=decay[:], in0=dec0[:], scalar1=htf[:, 0:1],
                                scalar2=None, op0=ALU.mult)
        eu = consts.tile([P, NCT], f32)
        nc.scalar.activation(out=eu[:], in_=ut[:], func=AF.Exp)
        c1 = consts.tile([P, NCT], f32)      # e^u - 1
        nc.vector.tensor_scalar(out=c1[:], in0=eu[:], scalar1=1.0, scalar2=None,
                                op0=ALU.subtract)

        # diag(c1) matrices for the PE-folded c1 scale, one per ctile
        diags = consts.tile([P, NCT, P], bf16)
        for j in range(NCT):
            nc.vector.tensor_scalar(out=diags[:, j, :], in0=ident[:],
                                    scalar1=c1[:, j:j + 1], scalar2=None,
                                    op0=ALU.mult)

        # DRAM views: (tc tp) (j cc) -> tp tc j cc
        def slab_src(ap, j):
            return ap.rearrange("(tc tp) (j cc) -> tp tc j cc", tp=P, cc=P)[:, :, j, :]

        for j in range(NCT):
            kslab = slabs.tile([P, NT, P], f32, tag="kslab")
            vslab = slabs.tile([P, NT, P], f32, tag="vslab")
            rslab = slabs.tile([P, NT, P], f32, tag="rslab")
            nc.sync.dma_start(out=kslab[:], in_=slab_src(k, j))
            nc.sync.dma_start(out=vslab[:], in_=slab_src(v, j))
            nc.sync.dma_start(out=rslab[:], in_=slab_src(r, j))

            ek = cbuf.tile([P, T_], bf16, tag="ek")
            ekv = cbuf.tile([P, T_], bf16, tag="ekv")
            for q in range(NCH):
                sl = slice(q * CH, (q + 1) * CH)
                pk = psin.tile([P, CH], f32, tag="pk")
                pv = psin.tile([P, CH], f32, tag="pv")
                for s in range(BPC):
                    tcb = q * BPC + s
                    bs = slice(s * P, (s + 1) * P)
                    nc.tensor.transpose(pk[:, bs], kslab[:, tcb, :], ident[:])
                    nc.tensor.transpose(pv[:, bs], vslab[:, tcb, :], ident[:])
                nc.scalar.activation(out=ek[:, sl], in_=pk[:], func=AF.Exp)
                nc.vector.tensor_tensor(ekv[:, sl], ek[:, sl], pv[:], ALU.mult)

            # ---- four scans, full T, broadcast per-channel decay ----
            djb = decay[:, j:j + 1].broadcast_to((P, T_))
            nf = scano.tile([P, T_], bf16, tag="nf")
            df = scano.tile([P, T_], bf16, tag="df")
            nb = scano.tile([P, T_], bf16, tag="nb")
            db = scano.tile([P, T_], bf16, tag="db")
            nc.vector.tensor_tensor_scan(out=nf[:], data0=djb, data1=ekv[:],
                                         initial=0.0, op0=ALU.mult, op1=ALU.add)
            nc.vector.tensor_tensor_scan(out=df[:], data0=djb, data1=ek[:],
                                         initial=0.0, op0=ALU.mult, op1=ALU.add)
            nc.vector.tensor_tensor_scan(out=nb[:, T_ - 1::-1], data0=djb,
                                         data1=ekv[:, T_ - 1::-1],
                                         initial=0.0, op0=ALU.mult, op1=ALU.add)
            nc.vector.tensor_tensor_scan(out=db[:, T_ - 1::-1], data0=djb,
                                         data1=ek[:, T_ - 1::-1],
                                         initial=0.0, op0=ALU.mult, op1=ALU.add)

            # ---- r-side in natural layout (no transpose) ----
            er = small.tile([P, NT, P], f32, tag="er")    # exp(-r), then inv
            l1p = small.tile([P, NT, P], f32, tag="l1p")  # ln(1+e^-r)
            lnd = small.tile([P, NT, P], f32, tag="lnd")  # ln(den), then s
            nc.scalar.activation(out=er[:], in_=rslab[:], func=AF.Exp, scale=-1.0)
            nc.scalar.activation(out=l1p[:], in_=er[:], func=AF.Ln, bias=1.0)

            oslab = slabs.tile([P, NT, P], f32, tag="oslab")
            dj = diags[:, j, :]
            for q in range(NCH):
                pnum = psnum.tile([P, CH], f32, tag="pnum")
                pden = psden.tile([P, CH], f32, tag="pden")
                for s in range(BPC):
                    tcb = q * BPC + s
                    bs = slice(s * P, (s + 1) * P)
                    tsl = slice(tcb * P, (tcb + 1) * P)
                    nc.tensor.matmul(pnum[:, bs], nf[:, tsl], identb[:],
                                     start=True, stop=False)
                    nc.tensor.matmul(pnum[:, bs], nb[:, tsl], identb[:],
                                     start=False, stop=False)
                    nc.tensor.matmul(pnum[:, bs], ekv[:, tsl], dj,
                                     start=False, stop=True)
                    nc.tensor.matmul(pden[:, bs], df[:, tsl], identb[:],
                                     start=True, stop=False)
                    nc.tensor.matmul(pden[:, bs], db[:, tsl], identb[:],
                                     start=False, stop=False)
                    nc.tensor.matmul(pden[:, bs], ek[:, tsl], dj,
                                     start=False, stop=True)
                qsl = slice(q * BPC, (q + 1) * BPC)
                # ln(den); s = ln(den)+l1p via gpsimd accum-DMA (keeps the
                # add off the DVE); inv = exp(-s+lnscale) in place
                nc.scalar.activation(out=lnd[:, qsl, :], in_=pden[:], func=AF.Ln)
                if j == NCT - 1:
                    # tail of the run: DVE is idle here, and the accum-DMA
                    # latency would sit exposed on the critical path
                    nc.vector.tensor_tensor(lnd[:, qsl, :], lnd[:, qsl, :],
                                            l1p[:, qsl, :], ALU.add)
                else:
                    nc.gpsimd.dma_start(out=lnd[:, qsl, :], in_=l1p[:, qsl, :],
                                        accum_op=ALU.add)
                nc.scalar.activation(out=er[:, qsl, :], in_=lnd[:, qsl, :],
                                     func=AF.Exp, scale=-1.0,
                                     bias=lnscale[:, 0:1])
                nc.vector.tensor_tensor(oslab[:, qsl, :], pnum[:],
                                        er[:, qsl, :], ALU.mult)
            nc.sync.dma_start(out=slab_src(out, j), in_=oslab[:])


def build_module(T_=T, C_=C, TD_=TD):
    nc = bacc.Bacc("TRN2", target_bir_lowering=False, debug=False)
    r = nc.dram_tensor("r", [T_, C_], f32, kind="ExternalInput").ap()
    k = nc.dram_tensor("k", [T_, C_], f32, kind="ExternalInput").ap()
    v = nc.dram_tensor("v", [T_, C_], f32, kind="ExternalInput").ap()
    w = nc.dram_tensor("w", [C_], f32, kind="ExternalInput").ap()
    u = nc.dram_tensor("u", [C_], f32, kind="ExternalInput").ap()
    te = nc.dram_tensor("time_emb", [TD_], f32, kind="ExternalInput").ap()
    out = nc.dram_tensor("out", [T_, C_], f32, kind="ExternalOutput").ap()
    with tile.TileContext(nc) as tc:
        _body(tc, out, r, k, v, w, u, te, T_, C_, TD_)
    nc.compile()
    return nc


_nc_cache = None


def run_full(r, k, v, w, u, time_emb, trace=False, **spmd_kwargs):
    """Run on 8 cores; returns (output [B,T,C], BassKernelResults)."""
    global _nc_cache
    if _nc_cache is None:
        _nc_cache = build_module()
    nc = _nc_cache
    r = np.asarray(r, dtype=np.float32)
    k = np.asarray(k, dtype=np.float32)
    v = np.asarray(v, dtype=np.float32)
    w = np.asarray(w, dtype=np.float32)
    u = np.asarray(u, dtype=np.float32)
    time_emb = np.asarray(time_emb, dtype=np.float32)
    in_maps = [
        {
            "r": np.ascontiguousarray(r[b]),
            "k": np.ascontiguousarray(k[b]),
            "v": np.ascontiguousarray(v[b]),
            "w": np.ascontiguousarray(w),
            "u": np.ascontiguousarray(u),
            "time_emb": np.ascontiguousarray(time_emb[b]),
        }
        for b in range(B)
    ]
    res = run_bass_kernel_spmd(nc, in_maps, core_ids=list(range(N_CORES)),
                               trace=trace, **spmd_kwargs)
    out = np.stack([res.results[b]["out"] for b in range(B)], axis=0)
    return out, res


def kernel(r, k, v, w, u, time_emb, **extra):
    out, _ = run_full(r, k, v, w, u, time_emb)
    return out


# revision 4
# speedup vs baseline: 1.0478x; 1.0478x over previous
"""Bi-directional WKV (RWKV-style) kernel for Trainium2, 8-core batch-parallel.

Math per (b, t, c):
    tf_b     = sigmoid(sum(time_emb[b]))
    decay_bc = exp(-exp(w_c)) * (0.5 + 0.5*tf_b)
    ek   = exp(k);  ekv = ek * v
    nf/df = forward inclusive scans of ekv/ek over t;  nb/db = backward
    num  = nf + nb + (e^u - 1)*ekv
    den  = df + db + (e^u - 1)*ek
    out  = sigmoid(r) * num/den * (0.8 + 0.2*tf_b)

Engine assignment (one batch element per core, [T=2048, C=2048] f32):

The DVE's four tensor_tensor_scan ops run at 2 cycles/elem regardless of
dtype (bf16 packed, materialized data0, Pool-engine offload: all measured
no-ops or compiler crashes), so everything else is pushed off the DVE:

  - num/den are assembled ON THE TENSOR ENGINE: per 128x128 block, three
    accumulating matmuls (start/stop flags) into one PSUM bank transpose
    nf/nb/ekv (df/db/ek) into natural [T, C] layout; the third uses a
    diag(e^u - 1) moving matrix, folding the c1 scale in for free.  bf16
    stationary data keeps these 1-pass (f32 matmul lowers to 2 passes).
    is_transpose=True cannot be used here: the transpose datapath ignores
    the moving matrix and cannot scale.
  - scan outputs and ek/ekv are written as bf16 (scan state stays fp32
    internally), halving their SBUF traffic and feeding the bf16 matmuls.
  - r is consumed in natural layout (never transposed); the sigmoid
    factor and the division run in log domain on the Scalar engine with
    one activation-table set (Ln/Exp only, zero table reloads):
       inv = exp(-(ln(den) + ln(1+e^-r)) + ln(0.8+0.2*tf))
  - the s = ln(den) + l1p add runs as a gpsimd SBUF->SBUF DMA with
    accum_op=add (software-DGE compute), not on the DVE.
  - log-domain tensors stay f32 (bf16's 0.4% relative error on ln(den)
    ~ 30 would exponentiate to ~12% output error).

DVE/ctile ends at 4 scans + 4 ekv mults + 4 final mults; the s-add runs
on the DVE for the last ctile only (DMA latency would sit exposed in the
tail there).  Wall 397us vs the 569us baseline: DVE busy 361us (268us is
the fixed scan rate), startup ~9us, drain ~8us; 13 measured overlap/
tiling perturbations (pool depths, emission reorders, CH=1024, bf16
conversion paths, finish deferral) all regressed vs this configuration.
"""

import numpy as np
from contextlib import ExitStack

import concourse.bass as bass
import concourse.bacc as bacc
import concourse.tile as tile
from concourse import mybir
from concourse.bass_utils import run_bass_kernel_spmd
from concourse.masks import make_identity

from concourse.hw_specs import get_activation_tables


def _pin_act_tables():
    # Confine the ACT table-set choice (see baseline note): every
    # Exp/Ln/Copy and the one Sigmoid can be served by two sets; hiding
    # the alternatives stops the table-load pass from bouncing.
    tabs = get_activation_tables("gen3")
    keep = {"natural_log_exp_and_others", "sigmoid_and_friends"}
    for name in list(tabs):
        if name not in keep:
            tabs[name] = set()


_pin_act_tables()

B, T, C, TD = 8, 2048, 2048, 512
P = 128
N_CORES = 8
f32 = mybir.dt.float32
bf16 = mybir.dt.bfloat16
ALU = mybir.AluOpType
AF = mybir.ActivationFunctionType


def _body(tc, out, r, k, v, w, u, te, T_, C_, TD_):
    nc = tc.nc
    NT = T_ // P   # t-blocks per ctile (16)
    NCT = C_ // P  # ctiles (16)
    CH = 512       # psum chunk width (one bank)
    BPC = CH // P  # 128-blocks per chunk (4)
    NCH = T_ // CH  # chunks per ctile (4)

    with ExitStack() as ctx:
        consts = ctx.enter_context(tc.tile_pool(name="consts", bufs=1))
        slabs = ctx.enter_context(tc.tile_pool(name="slabs", bufs=2))
        cbuf = ctx.enter_context(tc.tile_pool(name="cbuf", bufs=2))
        scano = ctx.enter_context(tc.tile_pool(name="scano", bufs=2))
        small = ctx.enter_context(tc.tile_pool(name="small", bufs=1))
        psin = ctx.enter_context(tc.tile_pool(name="psin", bufs=2, space="PSUM"))
        psnum = ctx.enter_context(tc.tile_pool(name="psnum", bufs=2, space="PSUM"))
        psden = ctx.enter_context(tc.tile_pool(name="psden", bufs=2, space="PSUM"))

        ident = consts.tile([P, P], f32)
        make_identity(nc, ident[:])
        identb = consts.tile([P, P], bf16)
        nc.vector.tensor_scalar(out=identb[:], in0=ident[:], scalar1=1.0,
                                scalar2=None, op0=ALU.mult)

        # ---- per-batch time factor on all partitions ----
        te_t = consts.tile([P, TD_], f32)
        te_b = bass.AP(tensor=te.tensor, offset=te.offset, ap=[[0, P]] + list(te.ap))
        nc.gpsimd.dma_start(out=te_t[:], in_=te_b)
        ssum = consts.tile([P, 1], f32)
        nc.vector.tensor_reduce(out=ssum[:], in_=te_t[:], axis=mybir.AxisListType.X,
                                op=ALU.add)
        tf = consts.tile([P, 1], f32)
        nc.scalar.activation(out=tf[:], in_=ssum[:], func=AF.Sigmoid)
        scale_b = consts.tile([P, 1], f32)   # 0.8 + 0.2*tf
        nc.vector.tensor_scalar(out=scale_b[:], in0=tf[:], scalar1=0.2, scalar2=0.8,
                                op0=ALU.mult, op1=ALU.add)
        htf = consts.tile([P, 1], f32)       # 0.5 + 0.5*tf
        nc.vector.tensor_scalar(out=htf[:], in0=tf[:], scalar1=0.5, scalar2=0.5,
                                op0=ALU.mult, op1=ALU.add)
        lnscale = consts.tile([P, 1], f32)   # ln(0.8 + 0.2*tf)
        nc.scalar.activation(out=lnscale[:], in_=scale_b[:], func=AF.Ln)

        # ---- per-channel constants [128, NCT]: channel = j*128 + p ----
        wt = consts.tile([P, NCT], f32)
        nc.gpsimd.dma_start(out=wt[:], in_=w.rearrange("(j p) -> p j", p=P))
        ut = consts.tile([P, NCT], f32)
        nc.gpsimd.dma_start(out=ut[:], in_=u.rearrange("(j p) -> p j", p=P))
        ew = consts.tile([P, NCT], f32)
        nc.scalar.activation(out=ew[:], in_=wt[:], func=AF.Exp)          # e^w
        dec0 = consts.tile([P, NCT], f32)
        nc.scalar.activation(out=dec0[:], in_=ew[:], func=AF.Exp, scale=-1.0)
        decay = consts.tile([P, NCT], f32)
        nc.vector.tensor_scalar(out=decay[:], in0=dec0[:], scalar1=htf[:, 0:1],
                                scalar2=None, op0=ALU.mult)
        eu = consts.tile([P, NCT], f32)
        nc.scalar.activation(out=eu[:], in_=ut[:], func=AF.Exp)
        c1 = consts.tile([P, NCT], f32)      # e^u - 1
        nc.vector.tensor_scalar(out=c1[:], in0=eu[:], scalar1=1.0, scalar2=None,
                                op0=ALU.subtract)

        # diag(c1) matrices for the PE-folded c1 scale, one per ctile
        diags = consts.tile([P, NCT, P], bf16)
        for j in range(NCT):
            nc.vector.tensor_scalar(out=diags[:, j, :], in0=ident[:],
                                    scalar1=c1[:, j:j + 1], scalar2=None,
                                    op0=ALU.mult)

        # DRAM views: (tc tp) (j cc) -> tp tc j cc
        def slab_src(ap, j):
            return ap.rearrange("(tc tp) (j cc) -> tp tc j cc", tp=P, cc=P)[:, :, j, :]

        for j in range(NCT):
            kslab = slabs.tile([P, NT, P], f32, tag="kslab")
            vslab = slabs.tile([P, NT, P], f32, tag="vslab")
            rslab = slabs.tile([P, NT, P], f32, tag="rslab")
            nc.sync.dma_start(out=kslab[:], in_=slab_src(k, j))
            nc.sync.dma_start(out=vslab[:], in_=slab_src(v, j))
            nc.sync.dma_start(out=rslab[:], in_=slab_src(r, j))

            ek = cbuf.tile([P, T_], bf16, tag="ek")
            ekv = cbuf.tile([P, T_], bf16, tag="ekv")
            for q in range(NCH):
                sl = slice(q * CH, (q + 1) * CH)
                pk = psin.tile([P, CH], f32, tag="pk")
                pv = psin.tile([P, CH], f32, tag="pv")
                for s in range(BPC):
                    tcb = q * BPC + s
                    bs = slice(s * P, (s + 1) * P)
                    nc.tensor.transpose(pk[:, bs], kslab[:, tcb, :], ident[:])
                    nc.tensor.transpose(pv[:, bs], vslab[:, tcb, :], ident[:])
                nc.scalar.activation(out=ek[:, sl], in_=pk[:], func=AF.Exp)
                nc.vector.tensor_tensor(ekv[:, sl], ek[:, sl], pv[:], ALU.mult)

            # ---- four scans, full T, broadcast per-channel decay ----
            djb = decay[:, j:j + 1].broadcast_to((P, T_))
            nf = scano.tile([P, T_], bf16, tag="nf")
            df = scano.tile([P, T_], bf16, tag="df")
            nb = scano.tile([P, T_], bf16, tag="nb")
            db = scano.tile([P, T_], bf16, tag="db")
            nc.vector.tensor_tensor_scan(out=nf[:], data0=djb, data1=ekv[:],
                                         initial=0.0, op0=ALU.mult, op1=ALU.add)
            nc.vector.tensor_tensor_scan(out=df[:], data0=djb, data1=ek[:],
                                         initial=0.0, op0=ALU.mult, op1=ALU.add)
            nc.vector.tensor_tensor_scan(out=nb[:, T_ - 1::-1], data0=djb,
                                         data1=ekv[:, T_ - 1::-1],
                                         initial=0.0, op0=ALU.mult, op1=ALU.add)
            nc.vector.tensor_tensor_scan(out=db[:, T_ - 1::-1], data0=djb,
                                         data1=ek[:, T_ - 1::-1],
                                         initial=0.0, op0=ALU.mult, op1=ALU.add)

            # ---- r-side in natural layout (no transpose) ----
            er = small.tile([P, NT, P], f32, tag="er")    # exp(-r), then inv
            l1p = small.tile([P, NT, P], f32, tag="l1p")  # ln(1+e^-r)
            lnd = small.tile([P, NT, P], f32, tag="lnd")  # ln(den), then s
            nc.scalar.activation(out=er[:], in_=rslab[:], func=AF.Exp, scale=-1.0)
            nc.scalar.activation(out=l1p[:], in_=er[:], func=AF.Ln, bias=1.0)

            oslab = slabs.tile([P, NT, P], f32, tag="oslab")
            dj = diags[:, j, :]
            for q in range(NCH):
                pnum = psnum.tile([P, CH], f32, tag="pnum")
                pden = psden.tile([P, CH], f32, tag="pden")
                for s in range(BPC):
                    tcb = q * BPC + s
                    bs = slice(s * P, (s + 1) * P)
                    tsl = slice(tcb * P, (tcb + 1) * P)
                    nc.tensor.matmul(pnum[:, bs], nf[:, tsl], identb[:],
                                     start=True, stop=False)
                    nc.tensor.matmul(pnum[:, bs], nb[:, tsl], identb[:],
                                     start=False, stop=False)
                    nc.tensor.matmul(pnum[:, bs], ekv[:, tsl], dj,
                                     start=False, stop=True)
                    nc.tensor.matmul(pden[:, bs], df[:, tsl], identb[:],
                                     start=True, stop=False)
                    nc.tensor.matmul(pden[:, bs], db[:, tsl], identb[:],
                                     start=False, stop=False)
                    nc.tensor.matmul(pden[:, bs], ek[:, tsl], dj,
                                     start=False, stop=True)
                qsl = slice(q * BPC, (q + 1) * BPC)
                # ln(den); s = ln(den)+l1p via gpsimd accum-DMA (keeps the
                # add off the DVE); inv = exp(-s+lnscale) in place
                nc.scalar.activation(out=lnd[:, qsl, :], in_=pden[:], func=AF.Ln)
                if j == NCT - 1:
                    # tail of the run: DVE is idle here, and the accum-DMA
                    # latency would sit exposed on the critical path
                    nc.vector.tensor_tensor(lnd[:, qsl, :], lnd[:, qsl, :],
                                            l1p[:, qsl, :], ALU.add)
                else:
                    nc.gpsimd.dma_start(out=lnd[:, qsl, :], in_=l1p[:, qsl, :],
                                        accum_op=ALU.add)
                nc.scalar.activation(out=er[:, qsl, :], in_=lnd[:, qsl, :],
                                     func=AF.Exp, scale=-1.0,
                                     bias=lnscale[:, 0:1])
                nc.vector.tensor_tensor(oslab[:, qsl, :], pnum[:],
                                        er[:, qsl, :], ALU.mult)
            nc.sync.dma_start(out=slab_src(out, j), in_=oslab[:])


def build_module(T_=T, C_=C, TD_=TD):
    nc = bacc.Bacc("TRN2", target_bir_lowering=False, debug=False)
    r = nc.dram_tensor("r", [T_, C_], f32, kind="ExternalInput").ap()
    k = nc.dram_tensor("k", [T_, C_], f32, kind="ExternalInput").ap()
    v = nc.dram_tensor("v", [T_, C_], f32, kind="ExternalInput").ap()
    w = nc.dram_tensor("w", [C_], f32, kind="ExternalInput").ap()
    u = nc.dram_tensor("u", [C_], f32, kind="ExternalInput").ap()
    te = nc.dram_tensor("time_emb", [TD_], f32, kind="ExternalInput").ap()
    out = nc.dram_tensor("out", [T_, C_], f32, kind="ExternalOutput").ap()
    with tile.TileContext(nc) as tc:
        _body(tc, out, r, k, v, w, u, te, T_, C_, TD_)
    nc.compile()
    return nc


_nc_cache = None


def run_full(r, k, v, w, u, time_emb, trace=False, **spmd_kwargs):
    """Run on 8 cores; returns (output [B,T,C], BassKernelResults)."""
    global _nc_cache
    if _nc_cache is None:
        _nc_cache = build_module()
    nc = _nc_cache
    r = np.asarray(r, dtype=np.float32)
    k = np.asarray(k, dtype=np.float32)
    v = np.asarray(v, dtype=np.float32)
    w = np.asarray(w, dtype=np.float32)
    u = np.asarray(u, dtype=np.float32)
    time_emb = np.asarray(time_emb, dtype=np.float32)
    in_maps = [
        {
            "r": np.ascontiguousarray(r[b]),
            "k": np.ascontiguousarray(k[b]),
            "v": np.ascontiguousarray(v[b]),
            "w": np.ascontiguousarray(w),
            "u": np.ascontiguousarray(u),
            "time_emb": np.ascontiguousarray(time_emb[b]),
        }
        for b in range(B)
    ]
    res = run_bass_kernel_spmd(nc, in_maps, core_ids=list(range(N_CORES)),
                               trace=trace, **spmd_kwargs)
    out = np.stack([res.results[b]["out"] for b in range(B)], axis=0)
    return out, res


def kernel(r, k, v, w, u, time_emb, **extra):
    out, _ = run_full(r, k, v, w, u, time_emb)
    return out
